# revision 38
# baseline (speedup 1.0000x reference)
"""Multi-head causal attention block on 8 TRN2 NeuronCores.

Strategy: 8-way tensor parallel over heads (2 heads/core, both batch rows
on every core). Mixed precision: fp16 for dense1 inputs / k / q / P / V
(PE runs fp16 at 1 cyc/row for any K, so the K=64 per-head score matmuls
go full rate and the two heads run concurrently in separate PE row
groups), fp16 for dense2, fp32 PSUM accumulation everywhere.

Per core:
  phase 1: project x -> kT/qT channel-major fp16 (heads stacked on
           partitions 0-63 / 64-127), v token-major fp16 via PE
           transpose of the channel-major result.
  phase 2: causal attention in S^T orientation (probabilities come out
           pre-transposed for the PV matmul — no transposes in the loop):
           S^T_h[j,i] = k_j . q_i  (1/sqrt(dh) folded into wq/bq on host)
           P^T = exp(S^T), exp evaluated only on the causally valid
           trapezoid (score matmuls, exp and PV all restrict their free
           dim to the valid columns of diagonal tiles; triangle band
           masked on DVE); both heads share one 2-bank PSUM tile so exp
           is a single wide op. O^T accumulated in PSUM with
           lhsT = [V_h | ones | pad]; PSUM row 64 = softmax denominator
           for free. PV runs two steps behind exp (software pipeline).
           At block end O^T/denominator are copied to SBUF immediately
           (frees PSUM); the normalize (DVE fast reciprocal, partition-
           broadcast via a zero-stride DMA through a DRAM bounce) is
           deferred into the next block's compute window.
  phase 3: fp16 AllToAll (8 ranks, 1MB/rank) turns per-core channel
           slices into per-core token slices; dense2 runs in out^T
           orientation (lhsT = W2 slices in fp16, rhs = received O^T
           slabs) so the bias (b2' = b1v @ W2 + b2, precomputed on host)
           is per-partition and applied for free in the PSUM drain
           activation.
Output: core c returns out^T of tokens [c%4*512:(c%4+1)*512) of batch
        c//4 as [128, 8, 512] (outch-in-tile, outch-tile, tok).
"""

import sys

if "/opt/trn_rl_repo" not in sys.path:
    sys.path.insert(0, "/opt/trn_rl_repo")

import numpy as np

import concourse.bass as bass
import concourse.mybir as mybir
import concourse.tile as tile
from concourse import bacc
from concourse.bass_utils import run_bass_kernel_spmd

F32 = mybir.dt.float32
F32R = mybir.dt.float32r
F16 = mybir.dt.float16
AF = mybir.ActivationFunctionType

B, T, D = 2, 2048, 1024
NHEADS, DH = 16, 64
NCORE = 8
TT = B * T            # 4096 global token rows
NCHUNK = 8            # 512-token chunks
NTILE = 32            # 128-token tiles
NBLK = 8              # 512-token attention/dense2 blocks


def build_nc():
    nc = bacc.Bacc(
        "TRN2",
        target_bir_lowering=False,
        debug=False,
        enable_asserts=True,
        num_devices=NCORE,
    )
    # ---- DRAM I/O (per core) ----
    xT_d = nc.dram_tensor("xT", [D, TT], F16, kind="ExternalInput")
    wk_d = nc.dram_tensor("wk", [128, 8, 128], F16, kind="ExternalInput")
    wq_d = nc.dram_tensor("wq", [128, 8, 128], F16, kind="ExternalInput")
    wv_d = nc.dram_tensor("wv", [128, 8, 128], F16, kind="ExternalInput")
    bk_d = nc.dram_tensor("bk", [128, 1], F32, kind="ExternalInput")
    bq_d = nc.dram_tensor("bq", [128, 1], F32, kind="ExternalInput")
    w2a_d = nc.dram_tensor("w2a", [128, 4, 1024], F16, kind="ExternalInput")
    w2b_d = nc.dram_tensor("w2b", [128, 4, 1024], F16, kind="ExternalInput")
    b2p_d = nc.dram_tensor("b2p", [128, 8], F32, kind="ExternalInput")
    masks_d = nc.dram_tensor("masks", [128, 256], F16, kind="ExternalInput")
    ones_d = nc.dram_tensor("ones", [1, 128], F32, kind="ExternalInput")
    ident_d = nc.dram_tensor("ident", [128, 128], F32R, kind="ExternalInput")
    out_d = nc.dram_tensor("out", [128, NBLK, 512], F16, kind="ExternalOutput")

    with tile.TileContext(nc) as tc, nc.allow_low_precision(reason="fp16 pipeline"):
        with (
            tc.tile_pool(name="const", bufs=1) as const,
            tc.tile_pool(name="kq", bufs=1) as kqp,
            tc.tile_pool(name="vp", bufs=1) as vp,
            tc.tile_pool(name="dram", bufs=1, space="DRAM") as dram,
        ):
            # ---- constants (bulky masks/w2 DMAs are emitted after phase 1
            # so they don't compete with the xT stream at kernel start) ----
            masks_sb = const.tile([128, 256], F16)
            onesf_sb = const.tile([1, 128], F32)
            ident_sb = const.tile([128, 128], F32R)
            w2a_sb = const.tile([128, 4, 1024], F16)
            w2b_sb = const.tile([128, 4, 1024], F16)
            b2p_sb = const.tile([128, 8], F32)
            bk_sb = const.tile([128, 1], F32)
            bq_sb = const.tile([128, 1], F32)

            # ---- persistent activations ----
            kT_sb = kqp.tile([128, TT], F16)     # rows 0-63 h0, 64-127 h1
            qT_sb = kqp.tile([128, TT], F16)
            # v token-major fp16, per 128-token tile: cols 0-63 V_h0,
            # 64 ones, 65-128 V_h1, 129 ones, 130-192 junk; PV lhsT is
            # padded to 128 cols ([65h, 65h+128)) — M=128 streams faster
            v_sb = vp.tile([128, NTILE, 193], F16)
            nc.gpsimd.memset(v_sb[:], 1.0)

            # ---- phase 1: projections ----
            with (
                tc.tile_pool(name="wslice", bufs=1) as wsl,
                tc.tile_pool(name="xin", bufs=3) as xin,
                tc.tile_pool(name="vtc", bufs=2) as vtc,
                tc.tile_pool(name="ps1", bufs=2, space="PSUM") as ps1,
                tc.tile_pool(name="pst", bufs=2, space="PSUM") as pst,
            ):
                wk_sb = wsl.tile([128, 8, 128], F16)
                wq_sb = wsl.tile([128, 8, 128], F16)
                wv_sb = wsl.tile([128, 8, 128], F16)
                # DMA descriptor issues are ~0.6us each, serial on Sync:
                # order them so the first matmul's inputs (wk slice 0,
                # x slice 0) are the first issues
                nc.sync.dma_start(wk_sb[:, 0:4, :], wk_d[:, 0:4, :])

                xT_r = xT_d.ap().rearrange("(a p) t -> p a t", p=128)

                def emit_transposes(vt_c, i8):
                    for t4 in range(4):
                        ps_tr = pst.tile([128, 128], F32R, tag="tp")
                        nc.tensor.transpose(ps_tr[:], vt_c[:, bass.ts(t4, 128)],
                                            ident_sb[:])
                        vi = i8 * 4 + t4
                        nc.vector.tensor_copy(v_sb[:, vi, 0:64], ps_tr[:, 0:64])
                        nc.vector.tensor_copy(v_sb[:, vi, 65:129], ps_tr[:, 64:128])

                pending_vt = None  # transpose chunk i8-1 during chunk i8's MMs
                for i8 in range(NCHUNK):
                    tsl = bass.ts(i8, 512)
                    # two half-chunk tiles so matmuls on the first half can
                    # start while the second half is still streaming in
                    xta = xin.tile([128, 4, 512], F16, tag="xta")
                    xtb = xin.tile([128, 4, 512], F16, tag="xtb")
                    if i8 == 0:
                        # first chunk: split + interleaved with the weight
                        # issues so the first matmuls' inputs issue first
                        nc.sync.dma_start(xta[:, 0:2, :], xT_r[:, 0:2, tsl])
                        nc.sync.dma_start(bk_sb[:], bk_d[:])
                        nc.sync.dma_start(wk_sb[:, 4:8, :], wk_d[:, 4:8, :])
                        nc.sync.dma_start(xta[:, 2:4, :], xT_r[:, 2:4, tsl])
                        nc.sync.dma_start(wq_sb[:], wq_d[:])
                        nc.sync.dma_start(xtb[:], xT_r[:, 4:8, tsl])
                        nc.sync.dma_start(wv_sb[:], wv_d[:])
                        nc.sync.dma_start(bq_sb[:], bq_d[:])
                        nc.sync.dma_start(ident_sb[:], ident_d[:])
                    else:
                        nc.sync.dma_start(xta[:], xT_r[:, 0:4, tsl])
                        nc.sync.dma_start(xtb[:], xT_r[:, 4:8, tsl])

                    def xt(a):
                        return xta[:, a, :] if a < 4 else xtb[:, a - 4, :]

                    # kT
                    psk = ps1.tile([128, 512], F32, tag="proj")
                    for a in range(8):
                        nc.tensor.matmul(psk[:], lhsT=wk_sb[:, a, :], rhs=xt(a),
                                         start=(a == 0), stop=(a == 7))
                    nc.scalar.activation(kT_sb[:, tsl], psk[:], AF.Identity,
                                         bias=bk_sb[:], scale=1.0)
                    # qT (wq/bq pre-scaled by 1/sqrt(dh) on host)
                    psq = ps1.tile([128, 512], F32, tag="proj")
                    for a in range(8):
                        nc.tensor.matmul(psq[:], lhsT=wq_sb[:, a, :], rhs=xt(a),
                                         start=(a == 0), stop=(a == 7))
                    nc.scalar.activation(qT_sb[:, tsl], psq[:], AF.Identity,
                                         bias=bq_sb[:], scale=1.0)
                    # vT (channel-major) then PE-transpose to token-major
                    psv = ps1.tile([128, 512], F32, tag="proj")
                    for a in range(8):
                        nc.tensor.matmul(psv[:], lhsT=wv_sb[:, a, :], rhs=xt(a),
                                         start=(a == 0), stop=(a == 7))
                    vt_c = vtc.tile([128, 512], F32R, tag="vt")
                    nc.scalar.activation(vt_c[:], psv[:], AF.Identity)
                    if pending_vt is not None:
                        emit_transposes(*pending_vt)
                    pending_vt = (vt_c, i8)
                emit_transposes(*pending_vt)

            # bulky constants for later phases — DMA'd while phase 1 computes
            nc.sync.dma_start(masks_sb[:], masks_d[:])
            nc.sync.dma_start(onesf_sb[:], ones_d[:])
            nc.sync.dma_start(w2a_sb[:], w2a_d[:])
            nc.sync.dma_start(w2b_sb[:], w2b_d[:])
            nc.sync.dma_start(b2p_sb[:], b2p_d[:])

            a2a_send0 = dram.tile([NBLK, 64, 512], F16)
            a2a_send1 = dram.tile([NBLK, 64, 512], F16)
            a2a_recv0 = dram.tile([NBLK, 64, 512], F16)
            a2a_recv1 = dram.tile([NBLK, 64, 512], F16)
            norm_dram = dram.tile([NBLK, 2, 512], F32)  # 1/denominator rows

            # ---- phase 2: attention ----
            with (
                tc.tile_pool(name="pp", bufs=6) as pp,
                tc.tile_pool(name="otp", bufs=2) as otp,
                tc.tile_pool(name="bcp", bufs=2) as bcp,
                tc.tile_pool(name="rcp", bufs=2) as rcp,
                tc.tile_pool(name="pss", bufs=2, space="PSUM") as pss,
                tc.tile_pool(name="pso", bufs=4, space="PSUM") as pso,
            ):
                def emit_pv(p_ap, h, blk_b, kj, po, nkj, off):
                    nc.tensor.matmul(
                        po[h][:, off:512],
                        lhsT=v_sb[:, blk_b * 16 + kj, 65 * h:65 * h + 128],
                        rhs=p_ap,
                        start=(kj == 0), stop=(kj == nkj - 1),
                        skip_group_check=True,
                    )

                def emit_copy(po, blk):
                    # pull O^T + denominator row (row 64) out of PSUM in one
                    # DVE copy per head right at block end so the po banks
                    # free quickly; the rest of the normalize is deferred
                    ous, dns = [], []
                    for h in range(2):
                        ou = otp.tile([65, 512], F32, tag="ou")
                        nc.vector.tensor_copy(ou[:], po[h][0:65, :])
                        dn = rcp.tile([1, 512], F32, tag="dn")
                        nc.vector.tensor_copy(dn[:], po[h][64:65, :])
                        ous.append(ou)
                        dns.append(dn)
                    return (ous, dns, blk)

                def emit_norm(ous, dns, blk, last=False):
                    # normalize O^T by 1/denominator, broadcast across 64
                    # partitions with a zero-stride DMA through a DRAM
                    # bounce (PE not involved). For the very last block the
                    # PE is idle, so a K=1 broadcast matmul (f32r mode, full
                    # rate) is lower latency than the DMA round-trip.
                    sends = [a2a_send0, a2a_send1]
                    for h in range(2):
                        rc = rcp.tile([1, 512], F32, tag="rc")
                        nc.vector.reciprocal_approx_fast(rc[:], dns[h][:])
                        bc = bcp.tile([64, 512], F32, tag="bcs")
                        if last:
                            pb = pso.tile([128, 512], F32, tag="o")
                            nc.tensor.matmul(pb[:], lhsT=onesf_sb[:, :],
                                             rhs=rc[:],
                                             start=True, stop=True)
                            nc.vector.tensor_copy(bc[:], pb[0:64, :])
                        else:
                            nc.sync.dma_start(norm_dram[blk, h], rc[:])
                            row = norm_dram[blk, h]
                            nc.sync.dma_start(
                                bc[:],
                                bass.AP(row.tensor, row.offset, [[0, 64], [1, 512]]))
                        ot = otp.tile([64, 512], F16, tag="ot")
                        nc.vector.tensor_mul(ot[:], ous[h][0:64, :], bc[:])
                        nc.sync.dma_start(sends[h][blk], ot[:])

                pending_copy = None   # (ous, dns, blk) awaiting normalize
                for b in range(B):
                    for qi in range(4):
                        blk = b * 4 + qi
                        qoff = b * T + qi * 512
                        nkj = 4 * qi + 4
                        po0 = pso.tile([128, 512], F32, tag="o")
                        po1 = pso.tile([128, 512], F32, tag="o")
                        po = [po0, po1]
                        pv_queue = []  # PV runs two kj behind S/exp
                        for kj in range(nkj):
                            koff = b * T + kj * 128
                            dp = kj - 4 * qi  # >=0: diagonal tile index
                            off = max(dp, 0) * 128  # first causally valid col
                            # both heads' score tiles in one 2-bank PSUM
                            # tile so exp is a single wide op; the two K=64
                            # fp16 matmuls sit in different PE row groups
                            # and execute concurrently
                            ss = pss.tile([128, 1024], F32, tag="s")
                            for h in range(2):
                                nc.tensor.matmul(
                                    ss[:, 512 * h + off:512 * h + 512],
                                    lhsT=kT_sb[64 * h:64 * h + 64, koff:koff + 128],
                                    rhs=qT_sb[64 * h:64 * h + 64,
                                              qoff + off:qoff + 512],
                                    start=True, stop=True,
                                )
                            p = pp.tile([128, 1024], F16, tag="p")
                            if off == 0:
                                nc.scalar.activation(p[:], ss[:], AF.Exp)
                            else:
                                # causal: cols < off are fully masked — the
                                # scores there were never computed; exp only
                                # the valid trapezoid
                                p_r = p[:].rearrange("q (h c) -> q h c", h=2)
                                s_r = ss[:].rearrange("q (h c) -> q h c", h=2)
                                nc.scalar.activation(p_r[:, :, off:512],
                                                     s_r[:, :, off:512], AF.Exp)
                            if dp >= 0:
                                # triangle band at the causal boundary
                                p_r = p[:].rearrange("q (h c) -> q h c", h=2)
                                m_r = masks_sb[:].rearrange("q (h c) -> q h c", h=2)
                                nc.vector.tensor_mul(
                                    p_r[:, :, off:off + 128],
                                    p_r[:, :, off:off + 128], m_r[:])
                            for h in range(2):
                                pv_queue.append(
                                    (p[:, 512 * h + off:512 * h + 512], h,
                                     b, kj, po, nkj, off))
                            if len(pv_queue) > 6:
                                emit_pv(*pv_queue.pop(0))
                                emit_pv(*pv_queue.pop(0))
                            if kj == 3 and pending_copy is not None:
                                emit_norm(*pending_copy)
                                pending_copy = None
                        for ppv in pv_queue:
                            emit_pv(*ppv)
                        if pending_copy is not None:
                            emit_norm(*pending_copy)
                            pending_copy = None
                        if blk < NBLK - 1:
                            pending_copy = emit_copy(po, blk)
                        else:
                            pending_po = (po, blk)
                # last block: no next block needs the po banks, so skip
                # the O^T SBUF copies and multiply straight out of PSUM;
                # broadcast 1/denom with a K=1 f32 matmul (PE idle here)
                po_last, blk_last = pending_po
                for h in range(2):
                    dn = rcp.tile([1, 512], F32, tag="dn")
                    nc.vector.tensor_copy(dn[:], po_last[h][64:65, :])
                    rc = rcp.tile([1, 512], F32, tag="rc")
                    nc.vector.reciprocal_approx_fast(rc[:], dn[:])
                    pb = pso.tile([128, 512], F32, tag="o")
                    nc.tensor.matmul(pb[:], lhsT=onesf_sb[:, :], rhs=rc[:],
                                     start=True, stop=True)
                    # bcast copy + multiply on GpSimd: pipelines with the
                    # other head's reciprocal on DVE
                    bc = bcp.tile([64, 512], F32, tag="bcs")
                    nc.vector.tensor_copy(bc[:], pb[0:64, :])
                    ot = otp.tile([64, 512], F16, tag="ot")
                    nc.vector.tensor_mul(ot[:], po_last[h][0:64, :], bc[:])
                    sends = [a2a_send0, a2a_send1]
                    nc.sync.dma_start(sends[h][blk_last], ot[:])

            # two half-sized AllToAlls (head 0 / head 1 channels): the
            # dense2 half-K accumulation for head 0 overlaps the second
            # collective's wire time
            nc.gpsimd.collective_compute(
                "AllToAll",
                mybir.AluOpType.bypass,
                replica_groups=[list(range(NCORE))],
                ins=[a2a_send0.opt()],
                outs=[a2a_recv0.opt()],
            )
            nc.gpsimd.collective_compute(
                "AllToAll",
                mybir.AluOpType.bypass,
                replica_groups=[list(range(NCORE))],
                ins=[a2a_send1.opt()],
                outs=[a2a_recv1.opt()],
            )

            # ---- phase 3: dense2 on own 512-token slice, out^T orientation
            # (lhsT = W2 slices, so the bias is per-partition and free).
            # Received slots are packed two per 128 partitions so the
            # matmuls keep K=128. ----
            with (
                tc.tile_pool(name="slb", bufs=1) as slb,
                tc.tile_pool(name="obp", bufs=4) as obp,
                tc.tile_pool(name="psd", bufs=8, space="PSUM") as psd,
            ):
                sl0 = slb.tile([128, 4, 512], F16)
                sl1 = slb.tile([128, 4, 512], F16)
                r0 = a2a_recv0[:].rearrange("(i r) p t -> r p i t", r=2)
                r1 = a2a_recv1[:].rearrange("(i r) p t -> r p i t", r=2)
                for half in range(2):
                    nc.sync.dma_start(sl0[0:64, 2 * half:2 * half + 2, :],
                                      r0[0][:, 2 * half:2 * half + 2, :])
                    nc.sync.dma_start(sl0[64:128, 2 * half:2 * half + 2, :],
                                      r0[1][:, 2 * half:2 * half + 2, :])
                    nc.sync.dma_start(sl1[0:64, 2 * half:2 * half + 2, :],
                                      r1[0][:, 2 * half:2 * half + 2, :])
                    nc.sync.dma_start(sl1[64:128, 2 * half:2 * half + 2, :],
                                      r1[1][:, 2 * half:2 * half + 2, :])
                pds = []
                for m in range(8):
                    pd = psd.tile([128, 512], F32, tag="d", name=f"pd{m}")
                    pds.append(pd)
                # slot-major order: the first matmuls only need the first
                # pair-DMA of each slab, so PE starts before the full slab
                # has landed
                for i in range(4):
                    for m in range(8):
                        nc.tensor.matmul(pds[m][:],
                                         lhsT=w2a_sb[:, i, bass.ts(m, 128)],
                                         rhs=sl0[:, i, :],
                                         start=(i == 0), stop=False,
                                         skip_group_check=True)
                ob = None
                for m in range(8):
                    for i in range(4):
                        nc.tensor.matmul(pds[m][:],
                                         lhsT=w2b_sb[:, i, bass.ts(m, 128)],
                                         rhs=sl1[:, i, :],
                                         start=False, stop=(i == 3),
                                         skip_group_check=True)
                    # drains alternate Scalar/DVE and pair into one tile so
                    # the out DMA needs only 4 issue slots (~0.6us each on
                    # the Sync engine) instead of 8-16
                    if m % 2 == 0:
                        ob = obp.tile([128, 2, 512], F16, tag="ob")
                        nc.scalar.activation(ob[:, 0, :], pds[m][:],
                                             AF.Identity,
                                             bias=b2p_sb[:, m:m + 1],
                                             scale=1.0)
                    else:
                        nc.vector.tensor_scalar_add(ob[:, 1, :], pds[m][:],
                                                    b2p_sb[:, m:m + 1])
                        nc.sync.dma_start(out_d[:, m - 1:m + 1, :], ob[:])

    nc.compile()
    return nc


_NC_CACHE = {}


def get_nc():
    if "nc" not in _NC_CACHE:
        _NC_CACHE["nc"] = build_nc()
    return _NC_CACHE["nc"]


def make_in_maps(x, W1, b1, W2, b2):
    x = np.asarray(x, dtype=np.float32)
    W1 = np.asarray(W1, dtype=np.float32)
    b1 = np.asarray(b1, dtype=np.float32)
    W2 = np.asarray(W2, dtype=np.float32)
    b2 = np.asarray(b2, dtype=np.float32)

    scale = np.float32(1.0 / np.sqrt(DH))
    xT = np.ascontiguousarray(x.reshape(TT, D).T)  # [D, TT]
    Wk, Wq, Wv = W1[:, :D], W1[:, D:2 * D], W1[:, 2 * D:]
    bk, bq, bv = b1[:D], b1[D:2 * D], b1[2 * D:]
    b2p = (bv @ W2 + b2).astype(np.float32)  # [D]

    # causal triangle band mask [128k x 128q], duplicated for both heads
    j = np.arange(128)[:, None]
    il = np.arange(128)[None, :]
    masks = np.tile((il >= j).astype(np.float32), (1, 2))

    ones = np.ones((1, 128), np.float32)
    ident = np.eye(128, dtype=np.float32)

    def stack(w):  # [1024, m] -> [128, 8, m] with [p, a, :] = w[a*128+p]
        return np.ascontiguousarray(
            w.reshape(8, 128, -1).transpose(1, 0, 2))

    xT16 = xT.astype(np.float16)
    masks16 = masks.astype(np.float16)
    W2r = W2.reshape(8, 2, 64, D)   # (slot, head, 64ch, outch)
    w2a16 = np.ascontiguousarray(
        W2r[:, 0].reshape(4, 128, D).transpose(1, 0, 2)).astype(np.float16)
    w2b16 = np.ascontiguousarray(
        W2r[:, 1].reshape(4, 128, D).transpose(1, 0, 2)).astype(np.float16)
    b2p_col = np.ascontiguousarray(b2p.reshape(8, 128).T)
    in_maps = []
    for c in range(NCORE):
        csl = slice(c * 128, (c + 1) * 128)
        in_maps.append({
            "xT": xT16,
            "wk": stack(Wk[:, csl]).astype(np.float16),
            "wq": stack(Wq[:, csl] * scale).astype(np.float16),
            "wv": stack(Wv[:, csl]).astype(np.float16),
            "bk": bk[csl].reshape(128, 1).copy(),
            "bq": (bq[csl] * scale).reshape(128, 1).copy(),
            "w2a": w2a16,
            "w2b": w2b16,
            "b2p": b2p_col,
            "masks": masks16,
            "ones": ones,
            "ident": ident,
        })
    return in_maps


def assemble(results):
    out = np.empty((B, T, D), dtype=np.float32)
    for c in range(NCORE):
        # res [128 outch-in-tile, 8 outch-tile, 512 tok] -> [tok, outch]
        b, t0 = c // 4, (c % 4) * 512
        out[b, t0:t0 + 512, :] = \
            results[c]["out"].transpose(2, 1, 0).reshape(512, D) \
            .astype(np.float32)
    return out


def kernel(x, W1, b1, W2, b2, _trace=False):
    nc = get_nc()
    in_maps = make_in_maps(x, W1, b1, W2, b2)
    kw = {"trace_cores": list(range(NCORE))} if _trace else {}
    res = run_bass_kernel_spmd(
        nc, in_maps, core_ids=list(range(NCORE)), trace=_trace, **kw)
    out = assemble(res.results)
    if _trace:
        return out, res
    return out

